# revision 12
# baseline (speedup 1.0000x reference)
"""Trainium2 Bass kernel for the attention-LSTM decoder step.

Computes, for B=256, S=512, H=512:
  energy  = tanh(enc @ Wa_e.T + h @ Wa_h.T + ba)      [B,S,H]
  scores  = energy @ v ; attn = softmax(scores)       [B,S]
  context = attn @ enc                                [B,H]
  LSTM single step on x=[input; context], then fc.

Sharding: data-parallel over batch across 8 NeuronCores (32 rows each);
all weights replicated.  Per core the energy matmul is computed as
energyT[g,s] per batch row (contraction over h on partitions, float32r
full-rate), with the per-(b,g) bias folded into the tanh activation.
Context is a fused multiply+reduce on the vector engine writing straight
into contextT columns for the LSTM stage.
"""

import numpy as np

import concourse.bass as bass
import concourse.tile as tile
from concourse import bacc, bass_utils, mybir

F32 = mybir.dt.float32
F32R = mybir.dt.float32r
BF16 = mybir.dt.bfloat16
AF = mybir.ActivationFunctionType
ALU = mybir.AluOpType

B, S, H = 256, 512, 512
NCORES = 8
BL = B // NCORES  # 32 batch rows per core
H4 = 4 * H

_CACHE = {}


def _r(ap):
    return ap.bitcast(F32R)


def _build(stage="full", blcap=BL):
    key = (stage, blcap)
    if key in _CACHE:
        return _CACHE[key]

    nc = bacc.Bacc(
        "TRN2",
        target_bir_lowering=False,
        debug=False,
        enable_asserts=False,
        num_devices=NCORES,
    )

    # ---- DRAM I/O ----
    enc_t = nc.dram_tensor("enc_t", [BL, H, S], BF16, kind="ExternalInput").ap()
    h_t = nc.dram_tensor("h_t", [H, BL], BF16, kind="ExternalInput").ap()
    in_t = nc.dram_tensor("in_t", [1, BL], BF16, kind="ExternalInput").ap()
    cell_n = nc.dram_tensor("cell_n", [BL, H], F32, kind="ExternalInput").ap()
    wae_t = nc.dram_tensor("wae_t", [H, H], BF16, kind="ExternalInput").ap()
    wah_t = nc.dram_tensor("wah_t", [H, H], BF16, kind="ExternalInput").ap()
    ba_c = nc.dram_tensor("ba_c", [H, 1], F32, kind="ExternalInput").ap()
    v_c = nc.dram_tensor("v_c", [H, 1], BF16, kind="ExternalInput").ap()
    wih_r0 = nc.dram_tensor("wih_r0", [1, H4], BF16, kind="ExternalInput").ap()
    wih_ctx = nc.dram_tensor("wih_ctx", [H, H4], BF16, kind="ExternalInput").ap()
    whh_t = nc.dram_tensor("whh_t", [H, H4], BF16, kind="ExternalInput").ap()
    bsum = nc.dram_tensor("bsum", [1, H4], BF16, kind="ExternalInput").ap()
    ones_d = nc.dram_tensor("ones_d", [1, 128], BF16, kind="ExternalInput").ap()
    fcw_b = nc.dram_tensor("fcw_b", [BL, H], F32, kind="ExternalInput").ap()
    fcb_c = nc.dram_tensor("fcb_c", [BL, 1], F32, kind="ExternalInput").ap()

    pred_o = nc.dram_tensor("pred_o", [BL, 1], F32, kind="ExternalOutput").ap()
    hnew_o = nc.dram_tensor("hnew_o", [BL, H], F32, kind="ExternalOutput").ap()
    cnew_o = nc.dram_tensor("cnew_o", [BL, H], F32, kind="ExternalOutput").ap()
    attn_o = nc.dram_tensor("attn_o", [BL, S], F32, kind="ExternalOutput").ap()

    with tile.TileContext(nc) as tc:
        with (
            tc.tile_pool(name="wpool", bufs=1) as wpool,
            tc.tile_pool(name="encp", bufs=12) as encp,
            tc.tile_pool(name="tanhp", bufs=8) as tanhp,
            tc.tile_pool(name="rows", bufs=4) as rows,
            tc.tile_pool(name="scr", bufs=2) as scr,
            tc.tile_pool(name="gatep", bufs=1) as gatep,
            tc.tile_pool(name="ppe", bufs=5, space="PSUM") as ppe,
            tc.tile_pool(name="pss", bufs=1, space="PSUM") as pss,
            tc.tile_pool(name="pab", bufs=2, space="PSUM") as pab,
        ):
            # ---- persistent weights / constants ----
            wae_sb = []
            wah_sb = []
            ht_sb = []
            ba_sb = []
            v_sb = []
            for kt in range(4):
                w1 = wpool.tile([128, H], BF16, tag=f"wae{kt}", name=f"wae{kt}")
                nc.sync.dma_start(w1[:], wae_t[kt * 128 : (kt + 1) * 128, :])
                wae_sb.append(w1)
                w2 = wpool.tile([128, H], BF16, tag=f"wah{kt}", name=f"wah{kt}")
                nc.sync.dma_start(w2[:], wah_t[kt * 128 : (kt + 1) * 128, :])
                wah_sb.append(w2)
                w3 = wpool.tile([128, BL], BF16, tag=f"ht{kt}", name=f"ht{kt}")
                nc.sync.dma_start(w3[:], h_t[kt * 128 : (kt + 1) * 128, :])
                ht_sb.append(w3)
                w4 = wpool.tile([128, 1], F32, tag=f"ba{kt}", name=f"ba{kt}")
                nc.sync.dma_start(w4[:], ba_c[kt * 128 : (kt + 1) * 128, :])
                ba_sb.append(w4)
                w5 = wpool.tile([128, 1], BF16, tag=f"v{kt}", name=f"v{kt}")
                nc.sync.dma_start(w5[:], v_c[kt * 128 : (kt + 1) * 128, :])
                v_sb.append(w5)

            int_sb = wpool.tile([1, BL], BF16, tag="int", name="int_sb")
            nc.sync.dma_start(int_sb[:], in_t[:])
            cell_sb = wpool.tile([BL, H], F32, tag="cell", name="cell_sb")
            nc.sync.dma_start(cell_sb[:], cell_n[:])
            fcwb_sb = wpool.tile([BL, H], F32, tag="fcwb", name="fcwb_sb")
            nc.sync.dma_start(fcwb_sb[:], fcw_b[:])
            fcb_sb = wpool.tile([BL, 1], F32, tag="fcb", name="fcb_sb")
            nc.sync.dma_start(fcb_sb[:], fcb_c[:])
            bsum_sb = wpool.tile([1, H4], BF16, tag="bsum", name="bsum_sb")
            nc.sync.dma_start(bsum_sb[:], bsum[:])
            wihr0_sb = wpool.tile([1, H4], BF16, tag="wihr0", name="wihr0_sb")
            nc.sync.dma_start(wihr0_sb[:], wih_r0[:])

            ones_col = wpool.tile([1, 128], BF16, tag="ones_c", name="ones_col")
            nc.sync.dma_start(ones_col[:], ones_d[:])
            ones_row = ones_col[:, :BL]

            # contextT columns, filled per batch row
            ctxT_sb = []
            biasT_sb = []
            for kt in range(4):
                t = wpool.tile([128, BL], BF16, tag=f"ctxT{kt}", name=f"ctxT{kt}")
                ctxT_sb.append(t)
                t = wpool.tile([128, BL], F32, tag=f"biasT{kt}", name=f"biasT{kt}")
                biasT_sb.append(t)

            # ---- stage 1: HhT = Wa_h @ h.T (+ba) -> per-partition tanh bias ----
            for gt in range(4):
                ph = ppe.tile([128, BL], F32, tag="pe", name=f"ph{gt}")
                for kt in range(4):
                    nc.tensor.matmul(
                        ph[:],
                        wah_sb[kt][:, gt * 128 : (gt + 1) * 128],
                        ht_sb[kt][:],
                        start=(kt == 0),
                        stop=(kt == 3),
                    )
                nc.vector.tensor_scalar_add(biasT_sb[gt][:], ph[:], ba_sb[gt][:])

            # ---- software-pipelined main loop over batch rows ----
            # iter b: DMA+energy+tanh(b); scores+softmax(b-1); bcast+context(b-2)
            enc_tiles = [None] * BL
            tanh_tiles = [None] * BL
            attn_rows = [None] * BL
            pab_tiles = [None] * BL

            # LSTM weights: emit DMAs after the first couple of enc prefetches
            wihc_sb = []
            whh_sb = []

            def emit_lstm_weight_dmas():
                for kt in range(4):
                    w = wpool.tile([128, H4], BF16, tag=f"wihc{kt}", name=f"wihc{kt}")
                    nc.sync.dma_start(w[:], wih_ctx[kt * 128 : (kt + 1) * 128, :])
                    wihc_sb.append(w)
                    w = wpool.tile([128, H4], BF16, tag=f"whh{kt}", name=f"whh{kt}")
                    nc.sync.dma_start(w[:], whh_t[kt * 128 : (kt + 1) * 128, :])
                    whh_sb.append(w)

            for it in range(blcap + 2):
                b = it
                if b < blcap:
                    # DMA encT for row b
                    etiles = []
                    for kt in range(4):
                        t = encp.tile([128, S], BF16, tag="enc", name=f"enc{b}_{kt}")
                        nc.sync.dma_start(
                            t[:], enc_t[b, kt * 128 : (kt + 1) * 128, :]
                        )
                        etiles.append(t)
                    enc_tiles[b] = etiles
                    if b == min(2, blcap - 1):
                        emit_lstm_weight_dmas()

                    # energyT = Wa_e @ encT (+bias) ; tanh
                    ttiles = []
                    for gt in range(4):
                        pe_ps = ppe.tile([128, S], F32, tag="pe", name=f"pe{b}_{gt}")
                        for kt in range(4):
                            nc.tensor.matmul(
                                pe_ps[:],
                                wae_sb[kt][:, gt * 128 : (gt + 1) * 128],
                                etiles[kt][:],
                                start=(kt == 0),
                                stop=(kt == 3),
                            )
                        tt = tanhp.tile([128, S], BF16, tag="tanh", name=f"th{b}_{gt}")
                        nc.scalar.activation(
                            tt[:],
                            pe_ps[:],
                            AF.Tanh,
                            bias=biasT_sb[gt][:, b : b + 1],
                        )
                        ttiles.append(tt)
                    tanh_tiles[b] = ttiles

                bs = it - 1
                if stage in ("scores", "bcast", "ttr", "ctx", "full") and 0 <= bs < blcap:
                    # scores = v.T @ tanhT ; softmax (unnormalized exp + 1/Z)
                    ps_ps = pss.tile([1, S], F32, tag="ps", name=f"ps{bs}")
                    for gt in range(4):
                        nc.tensor.matmul(
                            ps_ps[:],
                            v_sb[gt][:],
                            tanh_tiles[bs][gt][:],
                            start=(gt == 0),
                            stop=(gt == 3),
                        )
                    esc = rows.tile([1, S], F32, tag="esc", name=f"esc{bs}")
                    zz = rows.tile([1, 1], F32, tag="zz", name=f"zz{bs}")
                    nc.scalar.activation(esc[:], ps_ps[:], AF.Exp, accum_out=zz[:])
                    rz = rows.tile([1, 1], F32, tag="rz", name=f"rz{bs}")
                    nc.vector.reciprocal(rz[:], zz[:])
                    arow = rows.tile([1, S], F32, tag="arow", name=f"arow{bs}")
                    nc.vector.tensor_scalar_mul(arow[:], esc[:], rz[:])
                    nc.sync.dma_start(attn_o[bs : bs + 1, :], arow[:])
                    arow_b = rows.tile([1, S], BF16, tag="arowb", name=f"arowb{bs}")
                    nc.vector.tensor_scalar_mul(arow_b[:], esc[:], rz[:])
                    attn_rows[bs] = arow_b
                    tanh_tiles[bs] = None

                bc = it - 2
                if stage in ("bcast", "ttr", "ctx", "full") and 0 <= bc < blcap:
                    # broadcast attn row over 128 partitions via rank-1 matmul
                    pb = pab.tile([128, S], F32, tag="pab", name=f"pab{bc}")
                    nc.tensor.matmul(
                        pb[:],
                        ones_col[:],
                        attn_rows[bc][:],
                        start=True,
                        stop=True,
                    )
                    pab_tiles[bc] = pb
                    # contextT[:, bc] = sum_s encT * attn  (fused mul+reduce)
                    for ht in range(4 if stage != "bcast" else 0):
                        sc = scr.tile([128, S], F32, tag="ctxscr", name=f"cs{bc}_{ht}")
                        nc.vector.scalar_tensor_tensor(
                            out=sc[:],
                            in0=enc_tiles[bc][ht][:],
                            scalar=1.0,
                            in1=pb[:],
                            op0=ALU.mult,
                            op1=ALU.mult,
                            accum_out=ctxT_sb[ht][:, bc : bc + 1],
                        )
                    if stage == "bcast":
                        sc = scr.tile([128, S], F32, tag="ctxscr", name=f"cs{bc}")
                        nc.vector.tensor_copy(sc[:], pb[:])
                    elif stage == "ttr2":
                        pass
                    enc_tiles[bc] = None
                    attn_rows[bc] = None

            # ---- stage 4: LSTM gates ----
            if stage != "full":
                # dump bias + first tanh tiles so outputs exist
                dbg = gatep.tile([BL, H], F32, tag="dbg", name="dbg")
                nc.vector.memset(dbg[:], 0.0)
                nc.vector.tensor_copy(dbg[:, :BL], biasT_sb[0][:BL, :])
                nc.sync.dma_start(hnew_o[:], dbg[:])
                nc.sync.dma_start(cnew_o[:], dbg[:])
                dbg2 = gatep.tile([BL, 1], F32, tag="dbg2", name="dbg2")
                nc.vector.tensor_copy(dbg2[:], biasT_sb[0][:BL, 0:1])
                nc.sync.dma_start(pred_o[:], dbg2[:])
                if stage in ("bias", "energy"):
                    nc.sync.dma_start(attn_o[:], dbg[:, :S] if H >= S else dbg[:])
            gate_sb = []
            for ns in range(4 if stage == "full" else 0):
                pg = ppe.tile([BL, 512], F32, tag="pe", name=f"pg{ns}")
                nsl = slice(ns * 512, (ns + 1) * 512)
                nc.tensor.matmul(
                    pg[:], ones_row[:], bsum_sb[:, nsl], start=True, stop=False
                )
                nc.tensor.matmul(
                    pg[:], int_sb[:], wihr0_sb[:, nsl], start=False, stop=False
                )
                for kt in range(4):
                    nc.tensor.matmul(
                        pg[:],
                        ctxT_sb[kt][:],
                        wihc_sb[kt][:, nsl],
                        start=False,
                        stop=False,
                    )
                for kt in range(4):
                    nc.tensor.matmul(
                        pg[:],
                        ht_sb[kt][:],
                        whh_sb[kt][:, nsl],
                        start=False,
                        stop=(kt == 3),
                    )
                g = gatep.tile([BL, 512], F32, tag=f"gate{ns}", name=f"gate{ns}")
                nc.scalar.activation(
                    g[:], pg[:], AF.Tanh if ns == 2 else AF.Sigmoid
                )
                gate_sb.append(g)

            if stage == "full":
                fc_t = gatep.tile([BL, H], F32, tag="fc_t", name="fc_t")
                nc.vector.tensor_mul(fc_t[:], gate_sb[1][:], cell_sb[:])
                ig_t = gatep.tile([BL, H], F32, tag="ig_t", name="ig_t")
                nc.vector.tensor_mul(ig_t[:], gate_sb[0][:], gate_sb[2][:])
                cnew = gatep.tile([BL, H], F32, tag="cnew", name="cnew")
                nc.vector.tensor_add(cnew[:], fc_t[:], ig_t[:])
                nc.sync.dma_start(cnew_o[:], cnew[:])
                tnc = gatep.tile([BL, H], F32, tag="tnc", name="tnc")
                nc.scalar.activation(tnc[:], cnew[:], AF.Tanh)
                hnew = gatep.tile([BL, H], F32, tag="hnew", name="hnew")
                nc.vector.tensor_mul(hnew[:], gate_sb[3][:], tnc[:])
                nc.sync.dma_start(hnew_o[:], hnew[:])

                pscr = gatep.tile([BL, H], F32, tag="pscr", name="pscr")
                psum_fc = gatep.tile([BL, 1], F32, tag="psum_fc", name="psum_fc")
                nc.vector.scalar_tensor_tensor(
                    out=pscr[:],
                    in0=hnew[:],
                    scalar=1.0,
                    in1=fcwb_sb[:],
                    op0=ALU.mult,
                    op1=ALU.mult,
                    accum_out=psum_fc[:],
                )
                pred = gatep.tile([BL, 1], F32, tag="pred", name="pred")
                nc.vector.tensor_add(pred[:], psum_fc[:], fcb_sb[:])
                nc.sync.dma_start(pred_o[:], pred[:])

    nc.compile()
    _CACHE[key] = nc
    return nc


def make_in_maps(input, hidden, cell, encoder_outputs, Wa, ba, v, W_ih, W_hh, b_ih, b_hh, fc_W, fc_b):
    import ml_dtypes

    bf16 = ml_dtypes.bfloat16
    f = lambda x: np.ascontiguousarray(np.asarray(x, dtype=np.float32))
    input = f(input)
    h0 = f(hidden)[0]
    c0 = f(cell)[0]
    enc = f(encoder_outputs)
    Wa = f(Wa)
    ba = f(ba)
    v = f(v)
    W_ih = f(W_ih)
    W_hh = f(W_hh)
    b_ih = f(b_ih)
    b_hh = f(b_hh)
    fc_W = f(fc_W)
    fc_b = f(fc_b)

    enc_t_full = np.ascontiguousarray(enc.transpose(0, 2, 1))  # [B, H, S]
    shared = {
        "wae_t": np.ascontiguousarray(Wa[:, H:].T.astype(bf16)),  # [h, g]
        "wah_t": np.ascontiguousarray(Wa[:, :H].T.astype(bf16)),  # [k, g]
        "ba_c": np.ascontiguousarray(ba[:, None]),
        "v_c": np.ascontiguousarray(v[:, None].astype(bf16)),
        "wih_r0": np.ascontiguousarray(W_ih.T[0:1, :].astype(bf16)),  # [1, 4H]
        "wih_ctx": np.ascontiguousarray(W_ih.T[1:, :].astype(bf16)),  # [H, 4H]
        "whh_t": np.ascontiguousarray(W_hh.T.astype(bf16)),  # [H, 4H]
        "bsum": np.ascontiguousarray((b_ih + b_hh)[None, :].astype(bf16)),
        "fcw_b": np.ascontiguousarray(np.broadcast_to(fc_W[0][None, :], (BL, H))),
        "fcb_c": np.full((BL, 1), float(fc_b[0]), dtype=np.float32),
        "ones_d": np.ones((1, 128), dtype=bf16),
    }
    in_maps = []
    for c in range(NCORES):
        sl = slice(c * BL, (c + 1) * BL)
        m = dict(shared)
        m["enc_t"] = np.ascontiguousarray(enc_t_full[sl].astype(bf16))
        m["h_t"] = np.ascontiguousarray(h0[sl].T.astype(bf16))
        m["in_t"] = np.ascontiguousarray(input[sl].T.astype(bf16))
        m["cell_n"] = np.ascontiguousarray(c0[sl])
        in_maps.append(m)
    return in_maps


def run_sharded(in_maps, trace=False, trace_cores=None):
    nc = _build()
    return bass_utils.run_bass_kernel_spmd(
        nc,
        in_maps,
        core_ids=list(range(NCORES)),
        trace=trace,
        trace_cores=trace_cores,
    )


def kernel(**inputs):
    in_maps = make_in_maps(**inputs)
    res = run_sharded(in_maps)
    pred = np.concatenate([r["pred_o"] for r in res.results], axis=0)
    h_new = np.concatenate([r["hnew_o"] for r in res.results], axis=0)[None]
    c_new = np.concatenate([r["cnew_o"] for r in res.results], axis=0)[None]
    attn = np.concatenate([r["attn_o"] for r in res.results], axis=0)
    return (pred, h_new, c_new, attn)


# revision 13
# speedup vs baseline: 1.1741x; 1.1741x over previous
"""Trainium2 Bass kernel for the attention-LSTM decoder step.

Computes, for B=256, S=512, H=512:
  energy  = tanh(enc @ Wa_e.T + h @ Wa_h.T + ba)      [B,S,H]
  scores  = energy @ v ; attn = softmax(scores)       [B,S]
  context = attn @ enc                                [B,H]
  LSTM single step on x=[input; context], then fc prediction.

Sharding: data-parallel over batch across 8 NeuronCores (32 rows each);
weights replicated.  Per core the energy matmul is computed as
energyT[g,s] per batch row (contraction over h on partitions, bf16 at
full PE rate, fp32 PSUM accumulation), with the per-(b,g) bias folded
into the tanh activation's per-partition bias.  Context is a fused
multiply+reduce (scalar_tensor_tensor) on the vector engine writing
straight into contextT columns for the LSTM stage.  All contraction
k-tiles are folded side by side on the host so every tensor arrives in
one contiguous DMA.
"""

import numpy as np

import concourse.bass as bass
import concourse.tile as tile
from concourse import bacc, bass_utils, mybir

F32 = mybir.dt.float32
BF16 = mybir.dt.bfloat16
AF = mybir.ActivationFunctionType
ALU = mybir.AluOpType

B, S, H = 256, 512, 512
NCORES = 8
BL = B // NCORES  # 32 batch rows per core
H4 = 4 * H
KT = 4  # contraction tiles (H / 128)

_CACHE = {}


def _build(stage="full", blcap=BL):
    key = (stage, blcap)
    if key in _CACHE:
        return _CACHE[key]

    nc = bacc.Bacc(
        "TRN2",
        target_bir_lowering=False,
        debug=False,
        enable_asserts=False,
        num_devices=NCORES,
    )

    # ---- DRAM I/O (k-tiles pre-folded on host: [512, X] -> [128, 4*X]) ----
    enc_t = nc.dram_tensor("enc_t", [BL, 128, KT * S], BF16, kind="ExternalInput").ap()
    h_t = nc.dram_tensor("h_t", [128, KT * BL], BF16, kind="ExternalInput").ap()
    in_t = nc.dram_tensor("in_t", [1, BL], BF16, kind="ExternalInput").ap()
    cell_n = nc.dram_tensor("cell_n", [BL, H], F32, kind="ExternalInput").ap()
    wae_t = nc.dram_tensor("wae_t", [128, KT * H], BF16, kind="ExternalInput").ap()
    wah_t = nc.dram_tensor("wah_t", [128, KT * H], BF16, kind="ExternalInput").ap()
    ba_c = nc.dram_tensor("ba_c", [128, KT], F32, kind="ExternalInput").ap()
    v_c = nc.dram_tensor("v_c", [128, KT], BF16, kind="ExternalInput").ap()
    wih_r0 = nc.dram_tensor("wih_r0", [1, H4], BF16, kind="ExternalInput").ap()
    wih_ctx = nc.dram_tensor("wih_ctx", [128, KT * H4], BF16, kind="ExternalInput").ap()
    whh_t = nc.dram_tensor("whh_t", [128, KT * H4], BF16, kind="ExternalInput").ap()
    bsum = nc.dram_tensor("bsum", [1, H4], BF16, kind="ExternalInput").ap()
    ones_d = nc.dram_tensor("ones_d", [1, 128], BF16, kind="ExternalInput").ap()
    fcw_b = nc.dram_tensor("fcw_b", [BL, H], F32, kind="ExternalInput").ap()
    fcb_c = nc.dram_tensor("fcb_c", [BL, 1], F32, kind="ExternalInput").ap()

    pred_o = nc.dram_tensor("pred_o", [BL, 1], F32, kind="ExternalOutput").ap()
    hnew_o = nc.dram_tensor("hnew_o", [BL, H], F32, kind="ExternalOutput").ap()
    cnew_o = nc.dram_tensor("cnew_o", [BL, H], F32, kind="ExternalOutput").ap()
    attn_o = nc.dram_tensor("attn_o", [BL, S], F32, kind="ExternalOutput").ap()

    with tile.TileContext(nc) as tc:
        with (
            tc.tile_pool(name="wpool", bufs=1) as wpool,
            tc.tile_pool(name="encp", bufs=4) as encp,
            tc.tile_pool(name="tanhp", bufs=3) as tanhp,
            tc.tile_pool(name="rows", bufs=4) as rows,
            tc.tile_pool(name="scr", bufs=2) as scr,
            tc.tile_pool(name="gatep", bufs=1) as gatep,
            tc.tile_pool(name="ppe", bufs=5, space="PSUM") as ppe,
            tc.tile_pool(name="pss", bufs=1, space="PSUM") as pss,
            tc.tile_pool(name="pab", bufs=2, space="PSUM") as pab,
        ):
            # ---- weights, ordered by when the PE needs them ----
            wah_sb = wpool.tile([128, KT * H], BF16, tag="wah", name="wah_sb")
            nc.sync.dma_start(wah_sb[:], wah_t[:])
            ht_sb = wpool.tile([128, KT * BL], BF16, tag="ht", name="ht_sb")
            nc.sync.dma_start(ht_sb[:], h_t[:])
            ba_sb = wpool.tile([128, KT], F32, tag="ba", name="ba_sb")
            nc.sync.dma_start(ba_sb[:], ba_c[:])
            wae_sb = wpool.tile([128, KT * H], BF16, tag="wae", name="wae_sb")
            nc.sync.dma_start(wae_sb[:], wae_t[:])

            def wahs(kt, gt):  # lhsT [128, 128] for HhT
                return wah_sb[:, kt * H + gt * 128 : kt * H + (gt + 1) * 128]

            def waes(kt, gt):  # lhsT [128, 128] for energy
                return wae_sb[:, kt * H + gt * 128 : kt * H + (gt + 1) * 128]

            def hts(kt):  # [128, BL]
                return ht_sb[:, kt * BL : (kt + 1) * BL]

            # enc prefetch for the first rows happens before the small weights
            enc_tiles = [None] * BL

            def dma_enc(b):
                t = encp.tile([128, KT * S], BF16, tag="enc", name=f"enc{b}")
                nc.sync.dma_start(t[:], enc_t[b])
                enc_tiles[b] = t

            for b0 in range(min(3, blcap)):
                dma_enc(b0)

            v_sb = wpool.tile([128, KT], BF16, tag="v", name="v_sb")
            nc.sync.dma_start(v_sb[:], v_c[:])
            int_sb = wpool.tile([1, BL], BF16, tag="int", name="int_sb")
            nc.sync.dma_start(int_sb[:], in_t[:])
            ones_col = wpool.tile([1, 128], BF16, tag="ones_c", name="ones_col")
            nc.sync.dma_start(ones_col[:], ones_d[:])
            ones_row = ones_col[:, :BL]
            cell_sb = wpool.tile([BL, H], F32, tag="cell", name="cell_sb")
            nc.sync.dma_start(cell_sb[:], cell_n[:])
            fcwb_sb = wpool.tile([BL, H], F32, tag="fcwb", name="fcwb_sb")
            nc.sync.dma_start(fcwb_sb[:], fcw_b[:])
            fcb_sb = wpool.tile([BL, 1], F32, tag="fcb", name="fcb_sb")
            nc.sync.dma_start(fcb_sb[:], fcb_c[:])
            bsum_sb = wpool.tile([1, H4], BF16, tag="bsum", name="bsum_sb")
            nc.sync.dma_start(bsum_sb[:], bsum[:])
            wihr0_sb = wpool.tile([1, H4], BF16, tag="wihr0", name="wihr0_sb")
            nc.sync.dma_start(wihr0_sb[:], wih_r0[:])

            # contextT columns + tanh biases
            ctxT_sb = []
            biasT_sb = []
            for kt in range(KT):
                t = wpool.tile([128, BL], BF16, tag=f"ctxT{kt}", name=f"ctxT{kt}")
                ctxT_sb.append(t)
                t = wpool.tile([128, BL], F32, tag=f"biasT{kt}", name=f"biasT{kt}")
                biasT_sb.append(t)

            # ---- HhT = Wa_h @ h.T (+ba) -> per-partition tanh bias ----
            for gt in range(KT):
                ph = ppe.tile([128, BL], F32, tag="pe", name=f"ph{gt}")
                for kt in range(KT):
                    nc.tensor.matmul(
                        ph[:], wahs(kt, gt), hts(kt), start=(kt == 0), stop=(kt == 3)
                    )
                nc.vector.tensor_scalar_add(
                    biasT_sb[gt][:], ph[:], ba_sb[:, gt : gt + 1]
                )

            # LSTM weights (big, not needed until the tail): emitted mid-loop
            lstm_w = {}

            def emit_lstm_weight_dmas():
                w = wpool.tile([128, KT * H4], BF16, tag="wihc", name="wihc_sb")
                nc.sync.dma_start(w[:], wih_ctx[:])
                lstm_w["wihc"] = w
                w = wpool.tile([128, KT * H4], BF16, tag="whh", name="whh_sb")
                nc.sync.dma_start(w[:], whh_t[:])
                lstm_w["whh"] = w

            # ---- software-pipelined main loop over batch rows ----
            # iter i: energy+tanh(b=i); scores+softmax(i-1); bcast+context(i-2)
            tanh_tiles = [None] * BL
            attn_rows = [None] * BL

            for it in range(blcap + 2):
                b = it
                if b < blcap:
                    if b >= 3:
                        dma_enc(b)
                    if b == min(1, blcap - 1):
                        emit_lstm_weight_dmas()
                    et = enc_tiles[b]
                    tt = tanhp.tile([128, KT * S], BF16, tag="tanh", name=f"th{b}")
                    for gt in range(KT):
                        pe_ps = ppe.tile([128, S], F32, tag="pe", name=f"pe{b}_{gt}")
                        for kt in range(KT):
                            nc.tensor.matmul(
                                pe_ps[:],
                                waes(kt, gt),
                                et[:, kt * S : (kt + 1) * S],
                                start=(kt == 0),
                                stop=(kt == 3),
                            )
                        nc.scalar.activation(
                            tt[:, gt * S : (gt + 1) * S],
                            pe_ps[:],
                            AF.Tanh,
                            bias=biasT_sb[gt][:, b : b + 1],
                        )
                    tanh_tiles[b] = tt

                bs = it - 1
                if stage in ("scores", "ctx", "full") and 0 <= bs < blcap:
                    # scores = v.T @ tanhT ; softmax (exp + 1/Z; scores bounded)
                    ps_ps = pss.tile([1, S], F32, tag="ps", name=f"ps{bs}")
                    tts = tanh_tiles[bs]
                    for gt in range(KT):
                        nc.tensor.matmul(
                            ps_ps[:],
                            v_sb[:, gt : gt + 1],
                            tts[:, gt * S : (gt + 1) * S],
                            start=(gt == 0),
                            stop=(gt == 3),
                        )
                    esc = rows.tile([1, S], F32, tag="esc", name=f"esc{bs}")
                    zz = rows.tile([1, 1], F32, tag="zz", name=f"zz{bs}")
                    nc.scalar.activation(esc[:], ps_ps[:], AF.Exp, accum_out=zz[:])
                    rz = rows.tile([1, 1], F32, tag="rz", name=f"rz{bs}")
                    nc.vector.reciprocal(rz[:], zz[:])
                    arow = rows.tile([1, S], F32, tag="arow", name=f"arow{bs}")
                    nc.vector.tensor_scalar_mul(arow[:], esc[:], rz[:])
                    nc.gpsimd.dma_start(attn_o[bs : bs + 1, :], arow[:])
                    arow_b = rows.tile([1, S], BF16, tag="arowb", name=f"arowb{bs}")
                    nc.vector.tensor_scalar_mul(arow_b[:], esc[:], rz[:])
                    attn_rows[bs] = arow_b
                    tanh_tiles[bs] = None

                bc = it - 2
                if stage in ("ctx", "full") and 0 <= bc < blcap:
                    # broadcast attn row over 128 partitions via rank-1 matmul
                    pb = pab.tile([128, S], F32, tag="pab", name=f"pab{bc}")
                    nc.tensor.matmul(
                        pb[:], ones_col[:], attn_rows[bc][:], start=True, stop=True
                    )
                    # contextT[:, bc] = sum_s encT * attn  (fused mul+reduce)
                    for ht in range(KT):
                        sc = scr.tile([128, S], F32, tag="ctxscr", name=f"cs{bc}_{ht}")
                        nc.vector.scalar_tensor_tensor(
                            out=sc[:],
                            in0=enc_tiles[bc][:, ht * S : (ht + 1) * S],
                            scalar=1.0,
                            in1=pb[:],
                            op0=ALU.mult,
                            op1=ALU.mult,
                            accum_out=ctxT_sb[ht][:, bc : bc + 1],
                        )
                    enc_tiles[bc] = None
                    attn_rows[bc] = None

            # ---- LSTM gates + pointwise + fc ----
            if stage != "full":
                dbg = gatep.tile([BL, H], F32, tag="dbg", name="dbg")
                nc.vector.memset(dbg[:], 0.0)
                nc.vector.tensor_copy(dbg[:, :BL], biasT_sb[0][:BL, :])
                nc.sync.dma_start(hnew_o[:], dbg[:])
                nc.sync.dma_start(cnew_o[:], dbg[:])
                dbg2 = gatep.tile([BL, 1], F32, tag="dbg2", name="dbg2")
                nc.vector.tensor_copy(dbg2[:], biasT_sb[0][:BL, 0:1])
                nc.sync.dma_start(pred_o[:], dbg2[:])
                if stage in ("bias", "energy"):
                    nc.sync.dma_start(attn_o[:], dbg[:, :S])

            gate_sb = []
            for ns in range(4 if stage == "full" else 0):
                pg = ppe.tile([BL, 512], F32, tag="pe", name=f"pg{ns}")
                nsl = slice(ns * 512, (ns + 1) * 512)
                nc.tensor.matmul(
                    pg[:], ones_row[:], bsum_sb[:, nsl], start=True, stop=False
                )
                nc.tensor.matmul(
                    pg[:], int_sb[:], wihr0_sb[:, nsl], start=False, stop=False
                )
                for kt in range(KT):
                    nc.tensor.matmul(
                        pg[:],
                        ctxT_sb[kt][:],
                        lstm_w["wihc"][:, kt * H4 + ns * 512 : kt * H4 + (ns + 1) * 512],
                        start=False,
                        stop=False,
                    )
                for kt in range(KT):
                    nc.tensor.matmul(
                        pg[:],
                        hts(kt),
                        lstm_w["whh"][:, kt * H4 + ns * 512 : kt * H4 + (ns + 1) * 512],
                        start=False,
                        stop=(kt == 3),
                    )
                g = gatep.tile([BL, 512], F32, tag=f"gate{ns}", name=f"gate{ns}")
                nc.scalar.activation(g[:], pg[:], AF.Tanh if ns == 2 else AF.Sigmoid)
                gate_sb.append(g)

            if stage == "full":
                fc_t = gatep.tile([BL, H], F32, tag="fc_t", name="fc_t")
                nc.vector.tensor_mul(fc_t[:], gate_sb[1][:], cell_sb[:])
                ig_t = gatep.tile([BL, H], F32, tag="ig_t", name="ig_t")
                nc.vector.tensor_mul(ig_t[:], gate_sb[0][:], gate_sb[2][:])
                cnew = gatep.tile([BL, H], F32, tag="cnew", name="cnew")
                nc.vector.tensor_add(cnew[:], fc_t[:], ig_t[:])
                nc.sync.dma_start(cnew_o[:], cnew[:])
                tnc = gatep.tile([BL, H], F32, tag="tnc", name="tnc")
                nc.scalar.activation(tnc[:], cnew[:], AF.Tanh)
                hnew = gatep.tile([BL, H], F32, tag="hnew", name="hnew")
                nc.vector.tensor_mul(hnew[:], gate_sb[3][:], tnc[:])
                nc.sync.dma_start(hnew_o[:], hnew[:])

                pscr = gatep.tile([BL, H], F32, tag="pscr", name="pscr")
                psum_fc = gatep.tile([BL, 1], F32, tag="psum_fc", name="psum_fc")
                nc.vector.scalar_tensor_tensor(
                    out=pscr[:],
                    in0=hnew[:],
                    scalar=1.0,
                    in1=fcwb_sb[:],
                    op0=ALU.mult,
                    op1=ALU.mult,
                    accum_out=psum_fc[:],
                )
                pred = gatep.tile([BL, 1], F32, tag="pred", name="pred")
                nc.vector.tensor_add(pred[:], psum_fc[:], fcb_sb[:])
                nc.sync.dma_start(pred_o[:], pred[:])

    nc.compile()
    _CACHE[key] = nc
    return nc


def _fold_kt(a):
    """[512, X] -> [128, 4*X]: partition p holds rows p, p+128, p+256, p+384."""
    x = a.shape[1]
    return np.ascontiguousarray(
        a.reshape(KT, 128, x).transpose(1, 0, 2).reshape(128, KT * x)
    )


def make_in_maps(input, hidden, cell, encoder_outputs, Wa, ba, v, W_ih, W_hh, b_ih, b_hh, fc_W, fc_b):
    import ml_dtypes

    bf16 = ml_dtypes.bfloat16
    f = lambda x: np.ascontiguousarray(np.asarray(x, dtype=np.float32))
    input = f(input)
    h0 = f(hidden)[0]
    c0 = f(cell)[0]
    enc = f(encoder_outputs)
    Wa = f(Wa)
    ba = f(ba)
    v = f(v)
    W_ih = f(W_ih)
    W_hh = f(W_hh)
    b_ih = f(b_ih)
    b_hh = f(b_hh)
    fc_W = f(fc_W)
    fc_b = f(fc_b)

    # encT folded: [B, H, S] -> [B, 128, 4*S], bf16
    enc_bf = enc.astype(bf16)
    enc_t_full = np.ascontiguousarray(
        enc_bf.transpose(0, 2, 1)
        .reshape(B, KT, 128, S)
        .transpose(0, 2, 1, 3)
        .reshape(B, 128, KT * S)
    )

    shared = {
        "wae_t": _fold_kt(Wa[:, H:].T.astype(bf16)),
        "wah_t": _fold_kt(Wa[:, :H].T.astype(bf16)),
        "ba_c": _fold_kt(ba[:, None]),
        "v_c": _fold_kt(v[:, None].astype(bf16)),
        "wih_r0": np.ascontiguousarray(W_ih.T[0:1, :].astype(bf16)),
        "wih_ctx": _fold_kt(W_ih.T[1:, :].astype(bf16)),
        "whh_t": _fold_kt(W_hh.T.astype(bf16)),
        "bsum": np.ascontiguousarray((b_ih + b_hh)[None, :].astype(bf16)),
        "fcw_b": np.ascontiguousarray(np.broadcast_to(fc_W[0][None, :], (BL, H))),
        "fcb_c": np.full((BL, 1), float(fc_b[0]), dtype=np.float32),
        "ones_d": np.ones((1, 128), dtype=bf16),
    }
    in_maps = []
    for c in range(NCORES):
        sl = slice(c * BL, (c + 1) * BL)
        m = dict(shared)
        m["enc_t"] = enc_t_full[sl]
        m["h_t"] = _fold_kt(h0[sl].T.astype(bf16))
        m["in_t"] = np.ascontiguousarray(input[sl].T.astype(bf16))
        m["cell_n"] = np.ascontiguousarray(c0[sl])
        in_maps.append(m)
    return in_maps


def run_sharded(in_maps, trace=False, trace_cores=None):
    nc = _build()
    return bass_utils.run_bass_kernel_spmd(
        nc,
        in_maps,
        core_ids=list(range(NCORES)),
        trace=trace,
        trace_cores=trace_cores,
    )


def kernel(**inputs):
    in_maps = make_in_maps(**inputs)
    res = run_sharded(in_maps)
    pred = np.concatenate([r["pred_o"] for r in res.results], axis=0)
    h_new = np.concatenate([r["hnew_o"] for r in res.results], axis=0)[None]
    c_new = np.concatenate([r["cnew_o"] for r in res.results], axis=0)[None]
    attn = np.concatenate([r["attn_o"] for r in res.results], axis=0)
    return (pred, h_new, c_new, attn)


# revision 14
# speedup vs baseline: 1.2896x; 1.0984x over previous
"""Trainium2 Bass kernel for the attention-LSTM decoder step.

Computes, for B=256, S=512, H=512:
  energy  = tanh(enc @ Wa_e.T + h @ Wa_h.T + ba)      [B,S,H]
  scores  = energy @ v ; attn = softmax(scores)       [B,S]
  context = attn @ enc                                [B,H]
  LSTM single step on x=[input; context], then fc prediction.

Sharding: data-parallel over batch across 8 NeuronCores (32 rows each);
weights replicated.  Per core the energy matmul is computed as
energyT[g,s] per batch row (contraction over h on partitions, bf16 at
full PE rate, fp32 PSUM accumulation), with the per-(b,g) bias folded
into the tanh activation's per-partition bias.  Context is a fused
multiply+reduce (scalar_tensor_tensor) on the vector engine writing
straight into contextT columns for the LSTM stage.  All contraction
k-tiles are folded side by side on the host so every tensor arrives in
one contiguous DMA.
"""

import numpy as np

import concourse.bass as bass
import concourse.tile as tile
from concourse import bacc, bass_utils, mybir

F32 = mybir.dt.float32
BF16 = mybir.dt.bfloat16
AF = mybir.ActivationFunctionType
ALU = mybir.AluOpType

B, S, H = 256, 512, 512
NCORES = 8
BL = B // NCORES  # 32 batch rows per core
H4 = 4 * H
KT = 4  # contraction tiles (H / 128)

_CACHE = {}


def _build(stage="full", blcap=BL):
    key = (stage, blcap)
    if key in _CACHE:
        return _CACHE[key]

    nc = bacc.Bacc(
        "TRN2",
        target_bir_lowering=False,
        debug=False,
        enable_asserts=False,
        num_devices=NCORES,
    )

    # ---- DRAM I/O (k-tiles pre-folded on host: [512, X] -> [128, 4*X]) ----
    enc_t = nc.dram_tensor("enc_t", [BL, 128, KT * S], BF16, kind="ExternalInput").ap()
    h_t = nc.dram_tensor("h_t", [128, KT * BL], BF16, kind="ExternalInput").ap()
    in_t = nc.dram_tensor("in_t", [1, BL], BF16, kind="ExternalInput").ap()
    cell_n = nc.dram_tensor("cell_n", [BL, H], F32, kind="ExternalInput").ap()
    wae_t = nc.dram_tensor("wae_t", [128, KT * H], BF16, kind="ExternalInput").ap()
    wah_t = nc.dram_tensor("wah_t", [128, KT * H], BF16, kind="ExternalInput").ap()
    ba_c = nc.dram_tensor("ba_c", [128, KT], F32, kind="ExternalInput").ap()
    v_c = nc.dram_tensor("v_c", [128, KT], BF16, kind="ExternalInput").ap()
    wih_r0 = nc.dram_tensor("wih_r0", [1, H4], BF16, kind="ExternalInput").ap()
    wih_ctx = nc.dram_tensor("wih_ctx", [128, KT * H4], BF16, kind="ExternalInput").ap()
    whh_t = nc.dram_tensor("whh_t", [128, KT * H4], BF16, kind="ExternalInput").ap()
    bsum = nc.dram_tensor("bsum", [1, H4], BF16, kind="ExternalInput").ap()
    ones_d = nc.dram_tensor("ones_d", [1, 128], BF16, kind="ExternalInput").ap()
    fcw_b = nc.dram_tensor("fcw_b", [BL, H], F32, kind="ExternalInput").ap()
    fcb_c = nc.dram_tensor("fcb_c", [BL, 1], F32, kind="ExternalInput").ap()

    pred_o = nc.dram_tensor("pred_o", [BL, 1], F32, kind="ExternalOutput").ap()
    hnew_o = nc.dram_tensor("hnew_o", [BL, H], F32, kind="ExternalOutput").ap()
    cnew_o = nc.dram_tensor("cnew_o", [BL, H], F32, kind="ExternalOutput").ap()
    attn_o = nc.dram_tensor("attn_o", [BL, S], F32, kind="ExternalOutput").ap()

    with tile.TileContext(nc) as tc:
        with (
            tc.tile_pool(name="wpool", bufs=1) as wpool,
            tc.tile_pool(name="encp", bufs=6) as encp,
            tc.tile_pool(name="tanhp", bufs=3) as tanhp,
            tc.tile_pool(name="rows", bufs=4) as rows,
            tc.tile_pool(name="scr", bufs=2) as scr,
            tc.tile_pool(name="gatep", bufs=1) as gatep,
            tc.tile_pool(name="ppe", bufs=5, space="PSUM") as ppe,
            tc.tile_pool(name="pss", bufs=1, space="PSUM") as pss,
            tc.tile_pool(name="pab", bufs=2, space="PSUM") as pab,
        ):
            # ---- weights, ordered by when the PE needs them ----
            wae_sb = wpool.tile([128, KT * H], BF16, tag="wae", name="wae_sb")
            nc.sync.dma_start(wae_sb[:], wae_t[:])

            def wahs(kt, gt):  # lhsT [128, 128] for HhT
                return wah_sb[:, kt * H + gt * 128 : kt * H + (gt + 1) * 128]

            def waes(kt, gt):  # lhsT [128, 128] for energy
                return wae_sb[:, kt * H + gt * 128 : kt * H + (gt + 1) * 128]

            def hts(kt):  # [128, BL]
                return ht_sb[:, kt * BL : (kt + 1) * BL]

            # enc prefetch for the first rows happens before the small weights
            enc_tiles = [None] * BL

            def dma_enc(b):
                t = encp.tile([128, KT * S], BF16, tag="enc", name=f"enc{b}")
                nc.sync.dma_start(t[:], enc_t[b])
                enc_tiles[b] = t

            dma_enc(0)
            wah_sb = wpool.tile([128, KT * H], BF16, tag="wah", name="wah_sb")
            nc.sync.dma_start(wah_sb[:], wah_t[:])
            ht_sb = wpool.tile([128, KT * BL], BF16, tag="ht", name="ht_sb")
            nc.sync.dma_start(ht_sb[:], h_t[:])
            ba_sb = wpool.tile([128, KT], F32, tag="ba", name="ba_sb")
            nc.sync.dma_start(ba_sb[:], ba_c[:])
            for b0 in range(1, min(3, blcap)):
                dma_enc(b0)

            v_sb = wpool.tile([128, KT], BF16, tag="v", name="v_sb")
            nc.sync.dma_start(v_sb[:], v_c[:])
            int_sb = wpool.tile([1, BL], BF16, tag="int", name="int_sb")
            nc.sync.dma_start(int_sb[:], in_t[:])
            ones_col = wpool.tile([1, 128], BF16, tag="ones_c", name="ones_col")
            nc.sync.dma_start(ones_col[:], ones_d[:])
            ones_row = ones_col[:, :BL]
            cell_sb = wpool.tile([BL, H], F32, tag="cell", name="cell_sb")
            nc.sync.dma_start(cell_sb[:], cell_n[:])
            fcwb_sb = wpool.tile([BL, H], F32, tag="fcwb", name="fcwb_sb")
            nc.sync.dma_start(fcwb_sb[:], fcw_b[:])
            fcb_sb = wpool.tile([BL, 1], F32, tag="fcb", name="fcb_sb")
            nc.sync.dma_start(fcb_sb[:], fcb_c[:])
            bsum_sb = wpool.tile([1, H4], BF16, tag="bsum", name="bsum_sb")
            nc.sync.dma_start(bsum_sb[:], bsum[:])
            wihr0_sb = wpool.tile([1, H4], BF16, tag="wihr0", name="wihr0_sb")
            nc.sync.dma_start(wihr0_sb[:], wih_r0[:])

            # contextT columns + tanh biases
            ctxT_sb = []
            biasT_sb = []
            for kt in range(KT):
                t = wpool.tile([128, BL], BF16, tag=f"ctxT{kt}", name=f"ctxT{kt}")
                ctxT_sb.append(t)
                t = wpool.tile([128, BL], F32, tag=f"biasT{kt}", name=f"biasT{kt}")
                biasT_sb.append(t)

            def emit_hht():
                # HhT = Wa_h @ h.T (+ba) -> per-partition tanh bias
                for gt in range(KT):
                    ph = pab.tile([128, BL], F32, tag="pab", name=f"ph{gt}")
                    for kt in range(KT):
                        nc.tensor.matmul(
                            ph[:], wahs(kt, gt), hts(kt), start=(kt == 0), stop=(kt == 3)
                        )
                    nc.vector.tensor_scalar_add(
                        biasT_sb[gt][:], ph[:], ba_sb[:, gt : gt + 1]
                    )

            # LSTM weights (big, not needed until the tail): emitted mid-loop
            lstm_w = {}

            def emit_lstm_weight_dmas():
                w = wpool.tile([128, KT * H4], BF16, tag="wihc", name="wihc_sb")
                nc.sync.dma_start(w[:], wih_ctx[:])
                lstm_w["wihc"] = w
                w = wpool.tile([128, KT * H4], BF16, tag="whh", name="whh_sb")
                nc.sync.dma_start(w[:], whh_t[:])
                lstm_w["whh"] = w

            # ---- software-pipelined main loop over batch rows ----
            # iter i: energy+tanh(b=i); scores+softmax(i-1); bcast+context(i-2)
            tanh_tiles = [None] * BL
            attn_rows = [None] * BL

            for it in range(blcap + 2):
                b = it
                if b < blcap:
                    if b >= 3:
                        dma_enc(b)
                    if b == min(1, blcap - 1):
                        emit_lstm_weight_dmas()
                    et = enc_tiles[b]
                    tt = tanhp.tile([128, KT * S], BF16, tag="tanh", name=f"th{b}")
                    pend = []
                    for gt in range(KT):
                        pe_ps = ppe.tile([128, S], F32, tag="pe", name=f"pe{b}_{gt}")
                        for kt in range(KT):
                            nc.tensor.matmul(
                                pe_ps[:],
                                waes(kt, gt),
                                et[:, kt * S : (kt + 1) * S],
                                start=(kt == 0),
                                stop=(kt == 3),
                            )
                        if b == 0:
                            pend.append((gt, pe_ps))
                        else:
                            nc.scalar.activation(
                                tt[:, gt * S : (gt + 1) * S],
                                pe_ps[:],
                                AF.Tanh,
                                bias=biasT_sb[gt][:, b : b + 1],
                            )
                    if b == 0:
                        emit_hht()
                        for gt, pe_ps in pend:
                            nc.scalar.activation(
                                tt[:, gt * S : (gt + 1) * S],
                                pe_ps[:],
                                AF.Tanh,
                                bias=biasT_sb[gt][:, b : b + 1],
                            )
                    tanh_tiles[b] = tt

                bs = it - 1
                if stage in ("scores", "ctx", "full") and 0 <= bs < blcap:
                    # scores = v.T @ tanhT ; softmax (exp + 1/Z; scores bounded)
                    ps_ps = pss.tile([1, S], F32, tag="ps", name=f"ps{bs}")
                    tts = tanh_tiles[bs]
                    for gt in range(KT):
                        nc.tensor.matmul(
                            ps_ps[:],
                            v_sb[:, gt : gt + 1],
                            tts[:, gt * S : (gt + 1) * S],
                            start=(gt == 0),
                            stop=(gt == 3),
                        )
                    esc = rows.tile([1, S], F32, tag="esc", name=f"esc{bs}")
                    zz = rows.tile([1, 1], F32, tag="zz", name=f"zz{bs}")
                    nc.scalar.activation(esc[:], ps_ps[:], AF.Exp, accum_out=zz[:])
                    rz = rows.tile([1, 1], F32, tag="rz", name=f"rz{bs}")
                    nc.vector.reciprocal(rz[:], zz[:])
                    arow = rows.tile([1, S], F32, tag="arow", name=f"arow{bs}")
                    nc.vector.tensor_scalar_mul(arow[:], esc[:], rz[:])
                    nc.gpsimd.dma_start(attn_o[bs : bs + 1, :], arow[:])
                    arow_b = rows.tile([1, S], BF16, tag="arowb", name=f"arowb{bs}")
                    nc.vector.tensor_scalar_mul(arow_b[:], esc[:], rz[:])
                    attn_rows[bs] = arow_b
                    tanh_tiles[bs] = None

                bc = it - 2
                if stage in ("ctx", "full") and 0 <= bc < blcap:
                    # broadcast attn row over 128 partitions via rank-1 matmul
                    pb = pab.tile([128, S], F32, tag="pab", name=f"pab{bc}")
                    nc.tensor.matmul(
                        pb[:], ones_col[:], attn_rows[bc][:], start=True, stop=True
                    )
                    # contextT[:, bc] = sum_s encT * attn  (fused mul+reduce)
                    for ht in range(KT):
                        sc = scr.tile([128, S], F32, tag="ctxscr", name=f"cs{bc}_{ht}")
                        nc.vector.scalar_tensor_tensor(
                            out=sc[:],
                            in0=enc_tiles[bc][:, ht * S : (ht + 1) * S],
                            scalar=1.0,
                            in1=pb[:],
                            op0=ALU.mult,
                            op1=ALU.mult,
                            accum_out=ctxT_sb[ht][:, bc : bc + 1],
                        )
                    enc_tiles[bc] = None
                    attn_rows[bc] = None

            # ---- LSTM gates + pointwise + fc ----
            if stage != "full":
                dbg = gatep.tile([BL, H], F32, tag="dbg", name="dbg")
                nc.vector.memset(dbg[:], 0.0)
                nc.vector.tensor_copy(dbg[:, :BL], biasT_sb[0][:BL, :])
                nc.sync.dma_start(hnew_o[:], dbg[:])
                nc.sync.dma_start(cnew_o[:], dbg[:])
                dbg2 = gatep.tile([BL, 1], F32, tag="dbg2", name="dbg2")
                nc.vector.tensor_copy(dbg2[:], biasT_sb[0][:BL, 0:1])
                nc.sync.dma_start(pred_o[:], dbg2[:])
                if stage in ("bias", "energy"):
                    nc.sync.dma_start(attn_o[:], dbg[:, :S])

            gate_sb = []
            gate_ps = []
            for ns in range(4 if stage == "full" else 0):
                # static part first: bias + input + h@Whh (no ctx dependency)
                pg = ppe.tile([BL, 512], F32, tag="pe", name=f"pg{ns}")
                gate_ps.append(pg)
                nsl = slice(ns * 512, (ns + 1) * 512)
                nc.tensor.matmul(
                    pg[:], ones_row[:], bsum_sb[:, nsl], start=True, stop=False
                )
                nc.tensor.matmul(
                    pg[:], int_sb[:], wihr0_sb[:, nsl], start=False, stop=False
                )
                for kt in range(KT):
                    nc.tensor.matmul(
                        pg[:],
                        hts(kt),
                        lstm_w["whh"][:, kt * H4 + ns * 512 : kt * H4 + (ns + 1) * 512],
                        start=False,
                        stop=False,
                    )
            for ns in range(4 if stage == "full" else 0):
                pg = gate_ps[ns]
                for kt in range(KT):
                    nc.tensor.matmul(
                        pg[:],
                        ctxT_sb[kt][:],
                        lstm_w["wihc"][:, kt * H4 + ns * 512 : kt * H4 + (ns + 1) * 512],
                        start=False,
                        stop=(kt == 3),
                    )
                g = gatep.tile([BL, 512], F32, tag=f"gate{ns}", name=f"gate{ns}")
                nc.scalar.activation(g[:], pg[:], AF.Tanh if ns == 2 else AF.Sigmoid)
                gate_sb.append(g)

            if stage == "full":
                fc_t = gatep.tile([BL, H], F32, tag="fc_t", name="fc_t")
                nc.vector.tensor_mul(fc_t[:], gate_sb[1][:], cell_sb[:])
                ig_t = gatep.tile([BL, H], F32, tag="ig_t", name="ig_t")
                nc.vector.tensor_mul(ig_t[:], gate_sb[0][:], gate_sb[2][:])
                cnew = gatep.tile([BL, H], F32, tag="cnew", name="cnew")
                nc.vector.tensor_add(cnew[:], fc_t[:], ig_t[:])
                nc.sync.dma_start(cnew_o[:], cnew[:])
                tnc = gatep.tile([BL, H], F32, tag="tnc", name="tnc")
                nc.scalar.activation(tnc[:], cnew[:], AF.Tanh)
                hnew = gatep.tile([BL, H], F32, tag="hnew", name="hnew")
                nc.vector.tensor_mul(hnew[:], gate_sb[3][:], tnc[:])
                nc.sync.dma_start(hnew_o[:], hnew[:])

                pscr = gatep.tile([BL, H], F32, tag="pscr", name="pscr")
                psum_fc = gatep.tile([BL, 1], F32, tag="psum_fc", name="psum_fc")
                nc.vector.scalar_tensor_tensor(
                    out=pscr[:],
                    in0=hnew[:],
                    scalar=1.0,
                    in1=fcwb_sb[:],
                    op0=ALU.mult,
                    op1=ALU.mult,
                    accum_out=psum_fc[:],
                )
                pred = gatep.tile([BL, 1], F32, tag="pred", name="pred")
                nc.vector.tensor_add(pred[:], psum_fc[:], fcb_sb[:])
                nc.sync.dma_start(pred_o[:], pred[:])

    nc.compile()
    _CACHE[key] = nc
    return nc


def _fold_kt(a):
    """[512, X] -> [128, 4*X]: partition p holds rows p, p+128, p+256, p+384."""
    x = a.shape[1]
    return np.ascontiguousarray(
        a.reshape(KT, 128, x).transpose(1, 0, 2).reshape(128, KT * x)
    )


def make_in_maps(input, hidden, cell, encoder_outputs, Wa, ba, v, W_ih, W_hh, b_ih, b_hh, fc_W, fc_b):
    import ml_dtypes

    bf16 = ml_dtypes.bfloat16
    f = lambda x: np.ascontiguousarray(np.asarray(x, dtype=np.float32))
    input = f(input)
    h0 = f(hidden)[0]
    c0 = f(cell)[0]
    enc = f(encoder_outputs)
    Wa = f(Wa)
    ba = f(ba)
    v = f(v)
    W_ih = f(W_ih)
    W_hh = f(W_hh)
    b_ih = f(b_ih)
    b_hh = f(b_hh)
    fc_W = f(fc_W)
    fc_b = f(fc_b)

    # encT folded: [B, H, S] -> [B, 128, 4*S], bf16
    enc_bf = enc.astype(bf16)
    enc_t_full = np.ascontiguousarray(
        enc_bf.transpose(0, 2, 1)
        .reshape(B, KT, 128, S)
        .transpose(0, 2, 1, 3)
        .reshape(B, 128, KT * S)
    )

    shared = {
        "wae_t": _fold_kt(Wa[:, H:].T.astype(bf16)),
        "wah_t": _fold_kt(Wa[:, :H].T.astype(bf16)),
        "ba_c": _fold_kt(ba[:, None]),
        "v_c": _fold_kt(v[:, None].astype(bf16)),
        "wih_r0": np.ascontiguousarray(W_ih.T[0:1, :].astype(bf16)),
        "wih_ctx": _fold_kt(W_ih.T[1:, :].astype(bf16)),
        "whh_t": _fold_kt(W_hh.T.astype(bf16)),
        "bsum": np.ascontiguousarray((b_ih + b_hh)[None, :].astype(bf16)),
        "fcw_b": np.ascontiguousarray(np.broadcast_to(fc_W[0][None, :], (BL, H))),
        "fcb_c": np.full((BL, 1), float(fc_b[0]), dtype=np.float32),
        "ones_d": np.ones((1, 128), dtype=bf16),
    }
    in_maps = []
    for c in range(NCORES):
        sl = slice(c * BL, (c + 1) * BL)
        m = dict(shared)
        m["enc_t"] = enc_t_full[sl]
        m["h_t"] = _fold_kt(h0[sl].T.astype(bf16))
        m["in_t"] = np.ascontiguousarray(input[sl].T.astype(bf16))
        m["cell_n"] = np.ascontiguousarray(c0[sl])
        in_maps.append(m)
    return in_maps


def run_sharded(in_maps, trace=False, trace_cores=None):
    nc = _build()
    return bass_utils.run_bass_kernel_spmd(
        nc,
        in_maps,
        core_ids=list(range(NCORES)),
        trace=trace,
        trace_cores=trace_cores,
    )


def kernel(**inputs):
    in_maps = make_in_maps(**inputs)
    res = run_sharded(in_maps)
    pred = np.concatenate([r["pred_o"] for r in res.results], axis=0)
    h_new = np.concatenate([r["hnew_o"] for r in res.results], axis=0)[None]
    c_new = np.concatenate([r["cnew_o"] for r in res.results], axis=0)[None]
    attn = np.concatenate([r["attn_o"] for r in res.results], axis=0)
    return (pred, h_new, c_new, attn)


# revision 15
# speedup vs baseline: 1.3167x; 1.0210x over previous
"""Trainium2 Bass kernel for the attention-LSTM decoder step.

Computes, for B=256, S=512, H=512:
  energy  = tanh(enc @ Wa_e.T + h @ Wa_h.T + ba)      [B,S,H]
  scores  = energy @ v ; attn = softmax(scores)       [B,S]
  context = attn @ enc                                [B,H]
  LSTM single step on x=[input; context], then fc prediction.

Sharding: data-parallel over batch across 8 NeuronCores (32 rows each);
weights replicated.  Per core the energy matmul is computed as
energyT[g,s] per batch row (contraction over h on partitions, bf16 at
full PE rate, fp32 PSUM accumulation), with the per-(b,g) bias folded
into the tanh activation's per-partition bias.  Context is a fused
multiply+reduce (scalar_tensor_tensor) on the vector engine writing
straight into contextT columns for the LSTM stage.  All contraction
k-tiles are folded side by side on the host so every tensor arrives in
one contiguous DMA.
"""

import numpy as np

import concourse.bass as bass
import concourse.tile as tile
from concourse import bacc, bass_utils, mybir

F32 = mybir.dt.float32
BF16 = mybir.dt.bfloat16
AF = mybir.ActivationFunctionType
ALU = mybir.AluOpType

B, S, H = 256, 512, 512
NCORES = 8
BL = B // NCORES  # 32 batch rows per core
H4 = 4 * H
KT = 4  # contraction tiles (H / 128)

_CACHE = {}


def _build(stage="full", blcap=BL):
    key = (stage, blcap)
    if key in _CACHE:
        return _CACHE[key]

    nc = bacc.Bacc(
        "TRN2",
        target_bir_lowering=False,
        debug=False,
        enable_asserts=False,
        num_devices=NCORES,
    )

    # ---- DRAM I/O (k-tiles pre-folded on host: [512, X] -> [128, 4*X]) ----
    enc_t = nc.dram_tensor("enc_t", [BL, 128, KT * S], BF16, kind="ExternalInput").ap()
    h_t = nc.dram_tensor("h_t", [128, KT * BL], BF16, kind="ExternalInput").ap()
    in_t = nc.dram_tensor("in_t", [1, BL], BF16, kind="ExternalInput").ap()
    cell_n = nc.dram_tensor("cell_n", [BL, H], F32, kind="ExternalInput").ap()
    wae_t = nc.dram_tensor("wae_t", [128, KT * H], BF16, kind="ExternalInput").ap()
    wah_t = nc.dram_tensor("wah_t", [128, KT * H], BF16, kind="ExternalInput").ap()
    ba_c = nc.dram_tensor("ba_c", [128, KT], F32, kind="ExternalInput").ap()
    v_c = nc.dram_tensor("v_c", [128, KT], BF16, kind="ExternalInput").ap()
    wih_r0 = nc.dram_tensor("wih_r0", [1, H4], BF16, kind="ExternalInput").ap()
    wih_ctx = nc.dram_tensor("wih_ctx", [128, KT * H4], BF16, kind="ExternalInput").ap()
    whh_t = nc.dram_tensor("whh_t", [128, KT * H4], BF16, kind="ExternalInput").ap()
    bsum = nc.dram_tensor("bsum", [1, H4], BF16, kind="ExternalInput").ap()
    ones_d = nc.dram_tensor("ones_d", [1, 128], BF16, kind="ExternalInput").ap()
    fcw_b = nc.dram_tensor("fcw_b", [BL, H], F32, kind="ExternalInput").ap()
    fcb_c = nc.dram_tensor("fcb_c", [BL, 1], F32, kind="ExternalInput").ap()

    pred_o = nc.dram_tensor("pred_o", [BL, 1], F32, kind="ExternalOutput").ap()
    hnew_o = nc.dram_tensor("hnew_o", [BL, H], F32, kind="ExternalOutput").ap()
    cnew_o = nc.dram_tensor("cnew_o", [BL, H], F32, kind="ExternalOutput").ap()
    attn_o = nc.dram_tensor("attn_o", [BL, S], F32, kind="ExternalOutput").ap()

    with tile.TileContext(nc) as tc:
        with (
            tc.tile_pool(name="wpool", bufs=1) as wpool,
            tc.tile_pool(name="encp", bufs=8) as encp,
            tc.tile_pool(name="tanhp", bufs=3) as tanhp,
            tc.tile_pool(name="rows", bufs=4) as rows,
            tc.tile_pool(name="scr", bufs=2) as scr,
            tc.tile_pool(name="gatep", bufs=1) as gatep,
            tc.tile_pool(name="ppe", bufs=5, space="PSUM") as ppe,
            tc.tile_pool(name="pss", bufs=1, space="PSUM") as pss,
            tc.tile_pool(name="pab", bufs=2, space="PSUM") as pab,
        ):
            # ---- weights, ordered by when the PE needs them ----
            wae_sb = wpool.tile([128, KT * H], BF16, tag="wae", name="wae_sb")
            nc.sync.dma_start(wae_sb[:], wae_t[:])

            def wahs(kt, gt):  # lhsT [128, 128] for HhT
                return wah_sb[:, kt * H + gt * 128 : kt * H + (gt + 1) * 128]

            def waes(kt, gt):  # lhsT [128, 128] for energy
                return wae_sb[:, kt * H + gt * 128 : kt * H + (gt + 1) * 128]

            def hts(kt):  # [128, BL]
                return ht_sb[:, kt * BL : (kt + 1) * BL]

            # enc prefetch for the first rows happens before the small weights
            enc_tiles = [None] * BL

            def dma_enc(b, eng=None):
                t = encp.tile([128, KT * S], BF16, tag="enc", name=f"enc{b}")
                (eng or nc.sync).dma_start(t[:], enc_t[b])
                enc_tiles[b] = t

            dma_enc(0, nc.gpsimd)
            wah_sb = wpool.tile([128, KT * H], BF16, tag="wah", name="wah_sb")
            nc.sync.dma_start(wah_sb[:], wah_t[:])
            ht_sb = wpool.tile([128, KT * BL], BF16, tag="ht", name="ht_sb")
            nc.sync.dma_start(ht_sb[:], h_t[:])
            ba_sb = wpool.tile([128, KT], F32, tag="ba", name="ba_sb")
            nc.sync.dma_start(ba_sb[:], ba_c[:])
            for b0 in range(1, min(3, blcap)):
                dma_enc(b0, nc.gpsimd)

            v_sb = wpool.tile([128, KT], BF16, tag="v", name="v_sb")
            nc.sync.dma_start(v_sb[:], v_c[:])
            int_sb = wpool.tile([1, BL], BF16, tag="int", name="int_sb")
            nc.sync.dma_start(int_sb[:], in_t[:])
            ones_col = wpool.tile([1, 128], BF16, tag="ones_c", name="ones_col")
            nc.sync.dma_start(ones_col[:], ones_d[:])
            ones_row = ones_col[:, :BL]
            cell_sb = wpool.tile([BL, H], F32, tag="cell", name="cell_sb")
            nc.sync.dma_start(cell_sb[:], cell_n[:])
            fcwb_sb = wpool.tile([BL, H], F32, tag="fcwb", name="fcwb_sb")
            nc.sync.dma_start(fcwb_sb[:], fcw_b[:])
            fcb_sb = wpool.tile([BL, 1], F32, tag="fcb", name="fcb_sb")
            nc.sync.dma_start(fcb_sb[:], fcb_c[:])
            bsum_sb = wpool.tile([1, H4], BF16, tag="bsum", name="bsum_sb")
            nc.sync.dma_start(bsum_sb[:], bsum[:])
            wihr0_sb = wpool.tile([1, H4], BF16, tag="wihr0", name="wihr0_sb")
            nc.sync.dma_start(wihr0_sb[:], wih_r0[:])

            # contextT columns + tanh biases
            ctxT_sb = []
            biasT_sb = []
            for kt in range(KT):
                t = wpool.tile([128, BL], BF16, tag=f"ctxT{kt}", name=f"ctxT{kt}")
                ctxT_sb.append(t)
                t = wpool.tile([128, BL], F32, tag=f"biasT{kt}", name=f"biasT{kt}")
                biasT_sb.append(t)

            def emit_hht():
                # HhT = Wa_h @ h.T (+ba) -> per-partition tanh bias
                for gt in range(KT):
                    ph = pab.tile([128, BL], F32, tag="pab", name=f"ph{gt}")
                    for kt in range(KT):
                        nc.tensor.matmul(
                            ph[:], wahs(kt, gt), hts(kt), start=(kt == 0), stop=(kt == 3)
                        )
                    nc.vector.tensor_scalar_add(
                        biasT_sb[gt][:], ph[:], ba_sb[:, gt : gt + 1]
                    )

            # LSTM weights (big, not needed until the tail): emitted mid-loop
            lstm_w = {}

            def emit_lstm_weight_dmas():
                w = wpool.tile([128, KT * H4], BF16, tag="wihc", name="wihc_sb")
                nc.sync.dma_start(w[:], wih_ctx[:])
                lstm_w["wihc"] = w
                w = wpool.tile([128, KT * H4], BF16, tag="whh", name="whh_sb")
                nc.sync.dma_start(w[:], whh_t[:])
                lstm_w["whh"] = w

            # ---- software-pipelined main loop over batch rows ----
            # iter i: energy+tanh(b=i); scores+softmax(i-1); bcast+context(i-2)
            tanh_tiles = [None] * BL
            attn_rows = [None] * BL

            for it in range(blcap + 2):
                b = it
                if b < blcap:
                    if b + 3 < blcap:
                        dma_enc(b + 3)
                    elif b == 0 and blcap > 3:
                        dma_enc(3)
                        if blcap > 4:
                            dma_enc(4)
                    if b == min(1, blcap - 1):
                        emit_lstm_weight_dmas()
                    et = enc_tiles[b]
                    tt = tanhp.tile([128, KT * S], BF16, tag="tanh", name=f"th{b}")
                    pend = []
                    for gt in range(KT):
                        pe_ps = ppe.tile([128, S], F32, tag="pe", name=f"pe{b}_{gt}")
                        for kt in range(KT):
                            nc.tensor.matmul(
                                pe_ps[:],
                                waes(kt, gt),
                                et[:, kt * S : (kt + 1) * S],
                                start=(kt == 0),
                                stop=(kt == 3),
                            )
                        if b == 0:
                            pend.append((gt, pe_ps))
                        else:
                            nc.scalar.activation(
                                tt[:, gt * S : (gt + 1) * S],
                                pe_ps[:],
                                AF.Tanh,
                                bias=biasT_sb[gt][:, b : b + 1],
                            )
                    if b == 0:
                        emit_hht()
                        for gt, pe_ps in pend:
                            nc.scalar.activation(
                                tt[:, gt * S : (gt + 1) * S],
                                pe_ps[:],
                                AF.Tanh,
                                bias=biasT_sb[gt][:, b : b + 1],
                            )
                    tanh_tiles[b] = tt

                bs = it - 1
                if stage in ("scores", "ctx", "full") and 0 <= bs < blcap:
                    # scores = v.T @ tanhT ; softmax (exp + 1/Z; scores bounded)
                    ps_ps = pss.tile([1, S], F32, tag="ps", name=f"ps{bs}")
                    tts = tanh_tiles[bs]
                    for gt in range(KT):
                        nc.tensor.matmul(
                            ps_ps[:],
                            v_sb[:, gt : gt + 1],
                            tts[:, gt * S : (gt + 1) * S],
                            start=(gt == 0),
                            stop=(gt == 3),
                        )
                    esc = rows.tile([1, S], F32, tag="esc", name=f"esc{bs}")
                    zz = rows.tile([1, 1], F32, tag="zz", name=f"zz{bs}")
                    nc.scalar.activation(esc[:], ps_ps[:], AF.Exp, accum_out=zz[:])
                    rz = rows.tile([1, 1], F32, tag="rz", name=f"rz{bs}")
                    nc.vector.reciprocal(rz[:], zz[:])
                    arow = rows.tile([1, S], F32, tag="arow", name=f"arow{bs}")
                    nc.vector.tensor_scalar_mul(arow[:], esc[:], rz[:])
                    nc.gpsimd.dma_start(attn_o[bs : bs + 1, :], arow[:])
                    arow_b = rows.tile([1, S], BF16, tag="arowb", name=f"arowb{bs}")
                    nc.vector.tensor_scalar_mul(arow_b[:], esc[:], rz[:])
                    attn_rows[bs] = arow_b
                    tanh_tiles[bs] = None

                bc = it - 2
                if stage in ("ctx", "full") and 0 <= bc < blcap:
                    # broadcast attn row over 128 partitions via rank-1 matmul
                    pb = pab.tile([128, S], F32, tag="pab", name=f"pab{bc}")
                    nc.tensor.matmul(
                        pb[:], ones_col[:], attn_rows[bc][:], start=True, stop=True
                    )
                    # contextT[:, bc] = sum_s encT * attn  (fused mul+reduce)
                    for ht in range(KT):
                        sc = scr.tile([128, S], F32, tag="ctxscr", name=f"cs{bc}_{ht}")
                        nc.vector.scalar_tensor_tensor(
                            out=sc[:],
                            in0=enc_tiles[bc][:, ht * S : (ht + 1) * S],
                            scalar=1.0,
                            in1=pb[:],
                            op0=ALU.mult,
                            op1=ALU.mult,
                            accum_out=ctxT_sb[ht][:, bc : bc + 1],
                        )
                    enc_tiles[bc] = None
                    attn_rows[bc] = None

            # ---- LSTM gates + pointwise + fc ----
            if stage != "full":
                dbg = gatep.tile([BL, H], F32, tag="dbg", name="dbg")
                nc.vector.memset(dbg[:], 0.0)
                nc.vector.tensor_copy(dbg[:, :BL], biasT_sb[0][:BL, :])
                nc.sync.dma_start(hnew_o[:], dbg[:])
                nc.sync.dma_start(cnew_o[:], dbg[:])
                dbg2 = gatep.tile([BL, 1], F32, tag="dbg2", name="dbg2")
                nc.vector.tensor_copy(dbg2[:], biasT_sb[0][:BL, 0:1])
                nc.sync.dma_start(pred_o[:], dbg2[:])
                if stage in ("bias", "energy"):
                    nc.sync.dma_start(attn_o[:], dbg[:, :S])

            gate_sb = []
            gate_ps = []
            for ns in range(4 if stage == "full" else 0):
                # static part first: bias + input + h@Whh (no ctx dependency)
                pg = ppe.tile([BL, 512], F32, tag="pe", name=f"pg{ns}")
                gate_ps.append(pg)
                nsl = slice(ns * 512, (ns + 1) * 512)
                nc.tensor.matmul(
                    pg[:], ones_row[:], bsum_sb[:, nsl], start=True, stop=False
                )
                nc.tensor.matmul(
                    pg[:], int_sb[:], wihr0_sb[:, nsl], start=False, stop=False
                )
                for kt in range(KT):
                    nc.tensor.matmul(
                        pg[:],
                        hts(kt),
                        lstm_w["whh"][:, kt * H4 + ns * 512 : kt * H4 + (ns + 1) * 512],
                        start=False,
                        stop=False,
                    )
            for ns in range(4 if stage == "full" else 0):
                pg = gate_ps[ns]
                for kt in range(KT):
                    nc.tensor.matmul(
                        pg[:],
                        ctxT_sb[kt][:],
                        lstm_w["wihc"][:, kt * H4 + ns * 512 : kt * H4 + (ns + 1) * 512],
                        start=False,
                        stop=(kt == 3),
                    )
                g = gatep.tile([BL, 512], F32, tag=f"gate{ns}", name=f"gate{ns}")
                nc.scalar.activation(g[:], pg[:], AF.Tanh if ns == 2 else AF.Sigmoid)
                gate_sb.append(g)

            if stage == "full":
                fc_t = gatep.tile([BL, H], F32, tag="fc_t", name="fc_t")
                nc.vector.tensor_mul(fc_t[:], gate_sb[1][:], cell_sb[:])
                ig_t = gatep.tile([BL, H], F32, tag="ig_t", name="ig_t")
                nc.vector.tensor_mul(ig_t[:], gate_sb[0][:], gate_sb[2][:])
                cnew = gatep.tile([BL, H], F32, tag="cnew", name="cnew")
                nc.vector.tensor_add(cnew[:], fc_t[:], ig_t[:])
                nc.sync.dma_start(cnew_o[:], cnew[:])
                tnc = gatep.tile([BL, H], F32, tag="tnc", name="tnc")
                nc.scalar.activation(tnc[:], cnew[:], AF.Tanh)
                hnew = gatep.tile([BL, H], F32, tag="hnew", name="hnew")
                nc.vector.tensor_mul(hnew[:], gate_sb[3][:], tnc[:])
                nc.sync.dma_start(hnew_o[:], hnew[:])

                pscr = gatep.tile([BL, H], F32, tag="pscr", name="pscr")
                psum_fc = gatep.tile([BL, 1], F32, tag="psum_fc", name="psum_fc")
                nc.vector.scalar_tensor_tensor(
                    out=pscr[:],
                    in0=hnew[:],
                    scalar=1.0,
                    in1=fcwb_sb[:],
                    op0=ALU.mult,
                    op1=ALU.mult,
                    accum_out=psum_fc[:],
                )
                pred = gatep.tile([BL, 1], F32, tag="pred", name="pred")
                nc.vector.tensor_add(pred[:], psum_fc[:], fcb_sb[:])
                nc.sync.dma_start(pred_o[:], pred[:])

    nc.compile()
    _CACHE[key] = nc
    return nc


def _fold_kt(a):
    """[512, X] -> [128, 4*X]: partition p holds rows p, p+128, p+256, p+384."""
    x = a.shape[1]
    return np.ascontiguousarray(
        a.reshape(KT, 128, x).transpose(1, 0, 2).reshape(128, KT * x)
    )


def make_in_maps(input, hidden, cell, encoder_outputs, Wa, ba, v, W_ih, W_hh, b_ih, b_hh, fc_W, fc_b):
    import ml_dtypes

    bf16 = ml_dtypes.bfloat16
    f = lambda x: np.ascontiguousarray(np.asarray(x, dtype=np.float32))
    input = f(input)
    h0 = f(hidden)[0]
    c0 = f(cell)[0]
    enc = f(encoder_outputs)
    Wa = f(Wa)
    ba = f(ba)
    v = f(v)
    W_ih = f(W_ih)
    W_hh = f(W_hh)
    b_ih = f(b_ih)
    b_hh = f(b_hh)
    fc_W = f(fc_W)
    fc_b = f(fc_b)

    # encT folded: [B, H, S] -> [B, 128, 4*S], bf16
    enc_bf = enc.astype(bf16)
    enc_t_full = np.ascontiguousarray(
        enc_bf.transpose(0, 2, 1)
        .reshape(B, KT, 128, S)
        .transpose(0, 2, 1, 3)
        .reshape(B, 128, KT * S)
    )

    shared = {
        "wae_t": _fold_kt(Wa[:, H:].T.astype(bf16)),
        "wah_t": _fold_kt(Wa[:, :H].T.astype(bf16)),
        "ba_c": _fold_kt(ba[:, None]),
        "v_c": _fold_kt(v[:, None].astype(bf16)),
        "wih_r0": np.ascontiguousarray(W_ih.T[0:1, :].astype(bf16)),
        "wih_ctx": _fold_kt(W_ih.T[1:, :].astype(bf16)),
        "whh_t": _fold_kt(W_hh.T.astype(bf16)),
        "bsum": np.ascontiguousarray((b_ih + b_hh)[None, :].astype(bf16)),
        "fcw_b": np.ascontiguousarray(np.broadcast_to(fc_W[0][None, :], (BL, H))),
        "fcb_c": np.full((BL, 1), float(fc_b[0]), dtype=np.float32),
        "ones_d": np.ones((1, 128), dtype=bf16),
    }
    in_maps = []
    for c in range(NCORES):
        sl = slice(c * BL, (c + 1) * BL)
        m = dict(shared)
        m["enc_t"] = enc_t_full[sl]
        m["h_t"] = _fold_kt(h0[sl].T.astype(bf16))
        m["in_t"] = np.ascontiguousarray(input[sl].T.astype(bf16))
        m["cell_n"] = np.ascontiguousarray(c0[sl])
        in_maps.append(m)
    return in_maps


def run_sharded(in_maps, trace=False, trace_cores=None):
    nc = _build()
    return bass_utils.run_bass_kernel_spmd(
        nc,
        in_maps,
        core_ids=list(range(NCORES)),
        trace=trace,
        trace_cores=trace_cores,
    )


def kernel(**inputs):
    in_maps = make_in_maps(**inputs)
    res = run_sharded(in_maps)
    pred = np.concatenate([r["pred_o"] for r in res.results], axis=0)
    h_new = np.concatenate([r["hnew_o"] for r in res.results], axis=0)[None]
    c_new = np.concatenate([r["cnew_o"] for r in res.results], axis=0)[None]
    attn = np.concatenate([r["attn_o"] for r in res.results], axis=0)
    return (pred, h_new, c_new, attn)
